# revision 8
# baseline (speedup 1.0000x reference)
"""Distance-weighted self-attention on 8 Trainium2 NeuronCores.

The reference network is rank-1 in d_model and separable in the sequence:
  q = h*Wq, k = h*Wk, v = h*Wv  (h = heights column, sig = sizes column)
  logits[s,t] = c*h_s*h_t - 0.5*|sig_s - sig_t|,  c = (Wq.Wk)/16
  out[s,:]    = (num_s/den_s) * Wv,  num = sum_t h_t e^{L}, den = sum_t e^{L}

Two exact-enough structural reductions turn the O(S^2) attention into O(S):

1. |c*h_s*h_t| <= 0.05 for this input scale, so e^{c h_s h_t} is replaced
   by its 1st-order Taylor series in both num and den (the truncation
   errors largely cancel in the ratio; end-to-end rel err ~8e-5 vs the
   2e-2 gate, verified against the fp64 reference).
2. After sorting each row by sig (a host-side permutation, like the host
   transpose the previous kernel used; the inverse permutation is applied
   to the output rows on the host), e^{-0.5|sig_s - sig_t|} factorizes as
   e^{-sig_s/2} e^{+sig_t/2} for t <= s and the transpose for t >= s.
   With g_k = h^k e^{+sig/2}, f_k = h^k e^{-sig/2} (k = 0..2):
     A_k[s] = sum_t h_t^k e^{-0.5|sig_s-sig_t|}
            = e^{-sig_s/2}*prefix(g_k)[s] + e^{+sig_s/2}*suffix(f_k)[s]
              - h^k
     num = A_1 + (c h) A_2,   den = A_0 + (c h) A_1,   a = num/den

On device (one batch element per core, sorted order, layout [128, 16] with
element i on partition i//16): two ACT exps produce e^{+-sig/2}; four DVE
scalar_tensor_tensor ops produce g_1/g_2/f_1/f_2 with fused per-partition
totals; two DVE reduces total g_0/f_0; two tiny PE matmuls against
strict-triangular ones matrices turn the totals into cross-partition scan
offsets; six DVE tensor_tensor_scan ops (forward for g, reversed-AP for
f, offsets as the scan initial) give global prefix/suffix sums; a few
packed broadcast ops assemble a = num/den; the output rows a_s * Wv are
built [128, 256] at a time on DVE/ACT/gpsimd and DMAed out in five
chunks, the first after only two blocks so the serial DMA-engine phase
(2 MB at 360 GB/s ~ 5.8 us, the true floor) starts as early as possible.
"""

import os
import sys

import numpy as np

for _p in ("/opt/trn_rl_repo", "/root/.axon_site/_ro/trn_rl_repo"):
    if os.path.isdir(_p) and _p not in sys.path:
        sys.path.append(_p)

import concourse.bacc as bacc
import concourse.bass as bass
import concourse.masks as masks
import concourse.mybir as mybir
import concourse.tile as tile
from concourse.bass_utils import run_bass_kernel_spmd

S = 2048
D = 256
P = 128
NI = S // P  # 16 elements per partition, free-dim contiguous
N_CORES = 8

f32 = mybir.dt.float32
Alu = mybir.AluOpType
Act = mybir.ActivationFunctionType


def build_kernel(nc: bass.Bass, repeat: int = 1):
    # xcrit: host-packed per-partition layout [sig(16) | h(16) | wq(2) | wk(2)]
    # (sig/h sorted ascending by sig; element 16*p + i at [p, i]).
    xcrit = nc.dram_tensor("xcrit", [P, 2 * NI + 4], f32, kind="ExternalInput").ap()
    wvrep = nc.dram_tensor("wvrep", [P, D], f32, kind="ExternalInput").ap()
    out = nc.dram_tensor("out", [S, D], f32, kind="ExternalOutput").ap()

    with tile.TileContext(nc) as tc:
        from contextlib import ExitStack

        with ExitStack() as ctx:
            cpool = ctx.enter_context(tc.tile_pool(name="c", bufs=1))
            psum = ctx.enter_context(
                tc.tile_pool(name="ps", bufs=1, space=bass.MemorySpace.PSUM)
            )
            for _rep in range(repeat):
                _kernel_body(nc, tc, cpool, psum, xcrit, wvrep, out)
    return nc


def _kernel_body(nc, tc, cpool, psum, xcrit, wvrep, out):
    # ---- input DMAs (SP queue; xcrit first, it gates everything) --------
    xt = cpool.tile([P, 2 * NI + 4], f32)
    nc.sync.dma_start(xt[:], xcrit)
    wv_t = cpool.tile([P, D], f32)
    nc.sync.dma_start(wv_t[:], wvrep)
    sig = xt[:, 0:NI]
    h = xt[:, NI : 2 * NI]
    wq_t = xt[:, 2 * NI : 2 * NI + 2]
    wk_t = xt[:, 2 * NI + 2 : 2 * NI + 4]

    # ---- constants (no input dependency; hide under the DMA) -----------
    # Exp-table preload so the first real exp doesn't pay the 1.3us load.
    dummy = cpool.tile([P, 1], f32)
    nc.scalar.activation(dummy[:], dummy[:], Act.Exp)

    ones = cpool.tile([P, P], f32)
    nc.gpsimd.memset(ones[:], 1.0)
    # utri[p, m] = 1 where p < m (prefix offsets), ltri: p > m (suffix).
    utri = cpool.tile([P, P], f32)
    masks.make_upper_triangular(nc, utri[:], val=1.0, diag=False)
    ltri = cpool.tile([P, P], f32)
    masks.make_lower_triangular(nc, ltri[:], val=1.0, diag=False)
    # hpow[:, k, :] = h^k (k=0..2); ones part is input-independent.
    hpow = cpool.tile([P, 3, NI], f32)
    nc.gpsimd.memset(hpow[:, 0, :], 1.0)

    # ---- c = (Wq.Wk)/16 on every partition (off critical path) ---------
    wqk = cpool.tile([P, 2], f32)
    nc.gpsimd.tensor_mul(wqk[:], wq_t, wk_t)
    wred = cpool.tile([P, 1], f32)
    nc.vector.tensor_reduce(wred[:], wqk[:], axis=mybir.AxisListType.X, op=Alu.add)
    c_ps = psum.tile([P, 1], f32, tag="c")
    nc.tensor.matmul(c_ps[:], ones[:], wred[:], start=True, stop=True,
                     skip_group_check=True)
    c_sb = cpool.tile([P, 1], f32)

    # ---- h powers (gpsimd, parallel with the exps) ----------------------
    h2 = hpow[:, 2, :]
    nc.gpsimd.tensor_copy(hpow[:, 1, :], h)
    nc.gpsimd.tensor_mul(h2, h, h)

    # ---- e^{+-sig/2} and g_k/f_k with per-partition totals --------------
    # gpack[:, k, :] = h^k e^{+sig/2}, fpack[:, k, :] = h^k e^{-sig/2}
    gpack = cpool.tile([P, 3, NI], f32)
    fpack = cpool.tile([P, 3, NI], f32)
    tot = cpool.tile([P, 6], f32)  # [g0 g1 g2 f0 f1 f2]
    ep = gpack[:, 0, :]
    en = fpack[:, 0, :]
    nc.scalar.activation(ep, sig, Act.Exp, scale=0.5)
    nc.scalar.activation(en, sig, Act.Exp, scale=-0.5)
    # DVE chain, g-side first so the G offsets matmul can fire early.
    nc.vector.scalar_tensor_tensor(gpack[:, 1, :], h, 1.0, ep,
                                   op0=Alu.mult, op1=Alu.mult,
                                   accum_out=tot[:, 1:2])
    nc.vector.scalar_tensor_tensor(gpack[:, 2, :], h2, 1.0, ep,
                                   op0=Alu.mult, op1=Alu.mult,
                                   accum_out=tot[:, 2:3])
    nc.vector.tensor_reduce(tot[:, 0:1], ep, axis=mybir.AxisListType.X,
                            op=Alu.add)
    nc.vector.scalar_tensor_tensor(fpack[:, 1, :], h, 1.0, en,
                                   op0=Alu.mult, op1=Alu.mult,
                                   accum_out=tot[:, 4:5])
    nc.vector.scalar_tensor_tensor(fpack[:, 2, :], h2, 1.0, en,
                                   op0=Alu.mult, op1=Alu.mult,
                                   accum_out=tot[:, 5:6])
    nc.vector.tensor_reduce(tot[:, 3:4], en, axis=mybir.AxisListType.X,
                            op=Alu.add)

    # ---- cross-partition scan offsets via strict-triangular matmuls ----
    off_ps = psum.tile([P, 6], f32, tag="off")
    nc.tensor.matmul(off_ps[:, 0:3], utri[:], tot[:, 0:3], start=True,
                     stop=True, skip_group_check=True)
    nc.tensor.matmul(off_ps[:, 3:6], ltri[:], tot[:, 3:6], start=True,
                     stop=True, skip_group_check=True)
    # Split PSUM->SBUF copies on ACT so the G scans start before the F
    # offsets land; c_sb is scaled on ACT afterwards (it's only needed by
    # num/den, much later).
    offs = cpool.tile([P, 6], f32)
    nc.scalar.copy(offs[:, 0:3], off_ps[:, 0:3])
    nc.scalar.copy(offs[:, 3:6], off_ps[:, 3:6])
    nc.scalar.mul(c_sb[:], c_ps[:], 1.0 / 16.0)

    # ---- global prefix (g, forward) / suffix (f, reversed) scans --------
    scanG = cpool.tile([P, 3, NI], f32)
    scanF = cpool.tile([P, 3, NI], f32)
    for k in range(3):
        nc.vector.tensor_tensor_scan(
            scanG[:, k, :], gpack[:, k, :], gpack[:, k, :],
            initial=offs[:, k : k + 1], op0=Alu.add, op1=Alu.bypass,
        )
    # t1 = en * prefix while the F scans run.
    t1 = cpool.tile([P, 3, NI], f32)
    en_b = en.unsqueeze(1).broadcast_to([P, 3, NI])
    ep_b = ep.unsqueeze(1).broadcast_to([P, 3, NI])
    nc.vector.tensor_mul(t1[:], scanG[:], en_b)
    for k in range(3):
        nc.vector.tensor_tensor_scan(
            scanF[:, k, ::-1], fpack[:, k, ::-1], fpack[:, k, ::-1],
            initial=offs[:, 3 + k : 4 + k], op0=Alu.add, op1=Alu.bypass,
        )

    # ---- A_k = en*P_k + ep*Q_k - h^k; num/den/a -------------------------
    t2 = cpool.tile([P, 3, NI], f32)
    nc.vector.tensor_mul(t2[:], scanF[:], ep_b)
    s12 = cpool.tile([P, 3, NI], f32)
    nc.vector.tensor_add(s12[:], t1[:], t2[:])
    A = cpool.tile([P, 3, NI], f32)
    nc.vector.tensor_sub(A[:], s12[:], hpow[:])
    # m[:, 0, :] = h*A1, m[:, 1, :] = h*A2
    m = cpool.tile([P, 2, NI], f32)
    h_b = h.unsqueeze(1).broadcast_to([P, 2, NI])
    nc.vector.tensor_mul(m[:], A[:, 1:3, :], h_b)
    num = cpool.tile([P, NI], f32)
    nc.vector.scalar_tensor_tensor(num[:], m[:, 1, :], c_sb[:], A[:, 1, :],
                                   op0=Alu.mult, op1=Alu.add)
    den = cpool.tile([P, NI], f32)
    nc.vector.scalar_tensor_tensor(den[:], m[:, 0, :], c_sb[:], A[:, 0, :],
                                   op0=Alu.mult, op1=Alu.add)
    rden = cpool.tile([P, NI], f32)
    nc.vector.reciprocal(rden[:], den[:])
    a_t = cpool.tile([P, NI], f32)
    nc.vector.tensor_mul(a_t[:], num[:], rden[:])

    # ---- out rows: out[16p + i, :] = a[p, i] * Wv -----------------------
    # Chunks of [1, 1, 2, 4, 4, 4] blocks; the first DMA fires after one
    # block so the serial DMA-engine transfer phase (the floor) starts
    # ASAP. DVE blocks are the fastest (194 ns, 2x mode); ACT/Pool take a
    # few so chunk pacing always stays ahead of the transfer queue.
    out_sb = cpool.tile([P, NI, D], f32)
    out_r = out.rearrange("(p i) d -> p i d", p=P)
    chunks = [(0, 1), (1, 2), (2, 4), (4, 8), (8, 12), (12, 16)]
    eng_for = {2: "a", 4: "p", 6: "a", 8: "p", 10: "a", 12: "p", 14: "a"}
    for i in range(NI):
        dst = out_sb[:, i, :]
        a_col = a_t[:, i : i + 1]
        eng = eng_for.get(i, "v")
        if eng == "v":
            nc.vector.tensor_scalar_mul(dst, wv_t[:], a_col)
        elif eng == "a":
            nc.scalar.mul(dst, wv_t[:], a_col)
        else:
            nc.gpsimd.tensor_scalar_mul(dst, wv_t[:], a_col)
        for lo, hi in chunks:
            if i == hi - 1:
                nc.sync.dma_start(out_r[:, lo:hi, :], out_sb[:, lo:hi, :])


_NC = {}


def _get_nc(repeat: int = 1):
    if repeat not in _NC:
        nc = bacc.Bacc("TRN2", target_bir_lowering=False, debug=False,
                       num_devices=N_CORES)
        build_kernel(nc, repeat)
        nc.compile()
        _NC[repeat] = nc
    return _NC[repeat]


def kernel(inputs: np.ndarray, Wq: np.ndarray, Wk: np.ndarray, Wv: np.ndarray) -> np.ndarray:
    assert inputs.shape == (N_CORES, S, 2), inputs.shape
    nc = _get_nc()
    wq = np.asarray(Wq, dtype=np.float32).reshape(P, 2)
    wk = np.asarray(Wk, dtype=np.float32).reshape(P, 2)
    wvrep = np.ascontiguousarray(
        np.broadcast_to(np.asarray(Wv, dtype=np.float32).reshape(1, D), (P, D))
    )
    in_maps = []
    perms = []
    for b in range(N_CORES):
        sig = np.asarray(inputs[b, :, 0], dtype=np.float32)
        hgt = np.asarray(inputs[b, :, 1], dtype=np.float32)
        perm = np.argsort(sig, kind="stable")
        perms.append(perm)
        xcrit = np.empty((P, 2 * NI + 4), dtype=np.float32)
        xcrit[:, 0:NI] = sig[perm].reshape(P, NI)
        xcrit[:, NI : 2 * NI] = hgt[perm].reshape(P, NI)
        xcrit[:, 2 * NI : 2 * NI + 2] = wq
        xcrit[:, 2 * NI + 2 : 2 * NI + 4] = wk
        in_maps.append({"xcrit": xcrit, "wvrep": wvrep})
    res = run_bass_kernel_spmd(nc, in_maps, core_ids=list(range(N_CORES)))
    full = np.empty((N_CORES, S, D), dtype=np.float32)
    for b in range(N_CORES):
        inv = np.empty(S, dtype=np.int64)
        inv[perms[b]] = np.arange(S)
        full[b] = res.results[b]["out"][inv]
    return full


# revision 10
# speedup vs baseline: 1.0250x; 1.0250x over previous
"""Distance-weighted self-attention on 8 Trainium2 NeuronCores.

The reference network is rank-1 in d_model and separable in the sequence:
  q = h*Wq, k = h*Wk, v = h*Wv  (h = heights column, sig = sizes column)
  logits[s,t] = c*h_s*h_t - 0.5*|sig_s - sig_t|,  c = (Wq.Wk)/16
  out[s,:]    = (num_s/den_s) * Wv,  num = sum_t h_t e^{L}, den = sum_t e^{L}

Two exact-enough structural reductions turn the O(S^2) attention into O(S):

1. |c*h_s*h_t| <= 0.05 for this input scale, so e^{c h_s h_t} is replaced
   by its 1st-order Taylor series in both num and den (the truncation
   errors largely cancel in the ratio; end-to-end rel err ~8e-5 vs the
   2e-2 gate, verified against the fp64 reference).
2. After sorting each row by sig (a host-side permutation, like the host
   transpose the previous kernel used; the inverse permutation is applied
   to the output rows on the host), e^{-0.5|sig_s - sig_t|} factorizes as
   e^{-sig_s/2} e^{+sig_t/2} for t <= s and the transpose for t >= s.
   With g_k = h^k e^{+sig/2}, f_k = h^k e^{-sig/2} (k = 0..2):
     A_k[s] = sum_t h_t^k e^{-0.5|sig_s-sig_t|}
            = e^{-sig_s/2}*prefix(g_k)[s] + e^{+sig_s/2}*suffix(f_k)[s]
              - h^k
     num = A_1 + (c h) A_2,   den = A_0 + (c h) A_1,   a = num/den

On device (one batch element per core, sorted order, layout [128, 16] with
element i on partition i//16): two ACT exps produce e^{+-sig/2}; four DVE
scalar_tensor_tensor ops produce g_1/g_2/f_1/f_2 with fused per-partition
totals; two DVE reduces total g_0/f_0; two tiny PE matmuls against
strict-triangular ones matrices turn the totals into cross-partition scan
offsets; six DVE tensor_tensor_scan ops (forward for g, reversed-AP for
f, offsets as the scan initial) give global prefix/suffix sums; a few
packed broadcast ops assemble a = num/den; the output rows a_s * Wv are
built [128, 256] at a time on DVE/ACT/gpsimd and DMAed out in five
chunks, the first after only two blocks so the serial DMA-engine phase
(2 MB at 360 GB/s ~ 5.8 us, the true floor) starts as early as possible.
"""

import os
import sys

import numpy as np

for _p in ("/opt/trn_rl_repo", "/root/.axon_site/_ro/trn_rl_repo"):
    if os.path.isdir(_p) and _p not in sys.path:
        sys.path.append(_p)

import concourse.bacc as bacc
import concourse.bass as bass
import concourse.masks as masks
import concourse.mybir as mybir
import concourse.tile as tile
from concourse.bass_utils import run_bass_kernel_spmd

S = 2048
D = 256
P = 128
NI = S // P  # 16 elements per partition, free-dim contiguous
N_CORES = 8

f32 = mybir.dt.float32
Alu = mybir.AluOpType
Act = mybir.ActivationFunctionType


def build_kernel(nc: bass.Bass, repeat: int = 1):
    # xcrit: host-packed per-partition layout [sig(16) | h(16) | wq(2) | wk(2)]
    # (sig/h sorted ascending by sig; element 16*p + i at [p, i]).
    xcrit = nc.dram_tensor("xcrit", [P, 2 * NI + 4], f32, kind="ExternalInput").ap()
    wvrep = nc.dram_tensor("wvrep", [P, D], f32, kind="ExternalInput").ap()
    out = nc.dram_tensor("out", [S, D], f32, kind="ExternalOutput").ap()

    with tile.TileContext(nc) as tc:
        from contextlib import ExitStack

        with ExitStack() as ctx:
            cpool = ctx.enter_context(tc.tile_pool(name="c", bufs=1))
            psum = ctx.enter_context(
                tc.tile_pool(name="ps", bufs=1, space=bass.MemorySpace.PSUM)
            )
            for _rep in range(repeat):
                _kernel_body(nc, tc, cpool, psum, xcrit, wvrep, out)
    return nc


def _kernel_body(nc, tc, cpool, psum, xcrit, wvrep, out):
    # ---- input DMAs (SP queue; xcrit first, it gates everything) --------
    xt = cpool.tile([P, 2 * NI + 4], f32)
    nc.sync.dma_start(xt[:], xcrit)
    wv_t = cpool.tile([P, D], f32)
    nc.sync.dma_start(wv_t[:], wvrep)
    sig = xt[:, 0:NI]
    h = xt[:, NI : 2 * NI]
    wq_t = xt[:, 2 * NI : 2 * NI + 2]
    wk_t = xt[:, 2 * NI + 2 : 2 * NI + 4]

    # ---- constants (no input dependency; hide under the DMA) -----------
    # Exp-table preload so the first real exp doesn't pay the 1.3us load.
    dummy = cpool.tile([P, 1], f32)
    nc.scalar.activation(dummy[:], dummy[:], Act.Exp)

    ones = cpool.tile([P, P], f32)
    nc.gpsimd.memset(ones[:], 1.0)
    # utri[p, m] = 1 where p < m (prefix offsets), ltri: p > m (suffix).
    utri = cpool.tile([P, P], f32)
    masks.make_upper_triangular(nc, utri[:], val=1.0, diag=False)
    ltri = cpool.tile([P, P], f32)
    masks.make_lower_triangular(nc, ltri[:], val=1.0, diag=False)
    # hpow[:, k, :] = h^k (k=0..2); ones part is input-independent.
    hpow = cpool.tile([P, 3, NI], f32)
    nc.gpsimd.memset(hpow[:, 0, :], 1.0)

    # ---- c = (Wq.Wk)/16 on every partition (off critical path) ---------
    wqk = cpool.tile([P, 2], f32)
    nc.gpsimd.tensor_mul(wqk[:], wq_t, wk_t)
    wred = cpool.tile([P, 1], f32)
    nc.vector.tensor_reduce(wred[:], wqk[:], axis=mybir.AxisListType.X, op=Alu.add)
    c_ps = psum.tile([P, 1], f32, tag="c")
    nc.tensor.matmul(c_ps[:], ones[:], wred[:], start=True, stop=True,
                     skip_group_check=True)
    c_sb = cpool.tile([P, 1], f32)

    # ---- h powers (gpsimd, parallel with the exps) ----------------------
    h2 = hpow[:, 2, :]
    nc.gpsimd.tensor_copy(hpow[:, 1, :], h)
    nc.gpsimd.tensor_mul(h2, h, h)

    # ---- e^{+-sig/2} and g_k/f_k with per-partition totals --------------
    # gpack[:, k, :] = h^k e^{+sig/2}, fpack[:, k, :] = h^k e^{-sig/2}
    gpack = cpool.tile([P, 3, NI], f32)
    fpack = cpool.tile([P, 3, NI], f32)
    tot = cpool.tile([P, 6], f32)  # [g0 g1 g2 f0 f1 f2]
    ep = gpack[:, 0, :]
    en = fpack[:, 0, :]
    nc.scalar.activation(ep, sig, Act.Exp, scale=0.5)
    nc.scalar.activation(en, sig, Act.Exp, scale=-0.5)
    # DVE chain, g-side first so the G offsets matmul can fire early.
    nc.vector.scalar_tensor_tensor(gpack[:, 1, :], h, 1.0, ep,
                                   op0=Alu.mult, op1=Alu.mult,
                                   accum_out=tot[:, 1:2])
    nc.vector.scalar_tensor_tensor(gpack[:, 2, :], h2, 1.0, ep,
                                   op0=Alu.mult, op1=Alu.mult,
                                   accum_out=tot[:, 2:3])
    nc.vector.tensor_reduce(tot[:, 0:1], ep, axis=mybir.AxisListType.X,
                            op=Alu.add)
    nc.vector.scalar_tensor_tensor(fpack[:, 1, :], h, 1.0, en,
                                   op0=Alu.mult, op1=Alu.mult,
                                   accum_out=tot[:, 4:5])
    nc.vector.scalar_tensor_tensor(fpack[:, 2, :], h2, 1.0, en,
                                   op0=Alu.mult, op1=Alu.mult,
                                   accum_out=tot[:, 5:6])
    nc.vector.tensor_reduce(tot[:, 3:4], en, axis=mybir.AxisListType.X,
                            op=Alu.add)

    # ---- cross-partition scan offsets via strict-triangular matmuls ----
    off_ps = psum.tile([P, 6], f32, tag="off")
    nc.tensor.matmul(off_ps[:, 0:3], utri[:], tot[:, 0:3], start=True,
                     stop=True, skip_group_check=True)
    nc.tensor.matmul(off_ps[:, 3:6], ltri[:], tot[:, 3:6], start=True,
                     stop=True, skip_group_check=True)
    nc.scalar.mul(c_sb[:], c_ps[:], 1.0 / 16.0)

    # ---- global prefix (g, forward) / suffix (f, reversed) scans --------
    # The scan initial reads PSUM directly: ~125 ns extra per scan, but it
    # skips a PSUM->SBUF copy plus a PE->ACT->DVE semaphore chain.
    scanG = cpool.tile([P, 3, NI], f32)
    scanF = cpool.tile([P, 3, NI], f32)
    for k in range(3):
        nc.vector.tensor_tensor_scan(
            scanG[:, k, :], gpack[:, k, :], gpack[:, k, :],
            initial=off_ps[:, k : k + 1], op0=Alu.add, op1=Alu.bypass,
        )
    # t1 = en * prefix while the F scans run.
    t1 = cpool.tile([P, 3, NI], f32)
    en_b = en.unsqueeze(1).broadcast_to([P, 3, NI])
    ep_b = ep.unsqueeze(1).broadcast_to([P, 3, NI])
    nc.vector.tensor_mul(t1[:], scanG[:], en_b)
    for k in range(3):
        nc.vector.tensor_tensor_scan(
            scanF[:, k, ::-1], fpack[:, k, ::-1], fpack[:, k, ::-1],
            initial=off_ps[:, 3 + k : 4 + k], op0=Alu.add, op1=Alu.bypass,
        )

    # ---- A_k = en*P_k + ep*Q_k - h^k; num/den/a -------------------------
    t2 = cpool.tile([P, 3, NI], f32)
    nc.vector.tensor_mul(t2[:], scanF[:], ep_b)
    s12 = cpool.tile([P, 3, NI], f32)
    nc.vector.tensor_add(s12[:], t1[:], t2[:])
    A = cpool.tile([P, 3, NI], f32)
    nc.vector.tensor_sub(A[:], s12[:], hpow[:])
    # m[:, 0, :] = h*A1, m[:, 1, :] = h*A2
    m = cpool.tile([P, 2, NI], f32)
    h_b = h.unsqueeze(1).broadcast_to([P, 2, NI])
    nc.vector.tensor_mul(m[:], A[:, 1:3, :], h_b)
    num = cpool.tile([P, NI], f32)
    nc.vector.scalar_tensor_tensor(num[:], m[:, 1, :], c_sb[:], A[:, 1, :],
                                   op0=Alu.mult, op1=Alu.add)
    den = cpool.tile([P, NI], f32)
    nc.vector.scalar_tensor_tensor(den[:], m[:, 0, :], c_sb[:], A[:, 0, :],
                                   op0=Alu.mult, op1=Alu.add)
    rden = cpool.tile([P, NI], f32)
    nc.vector.reciprocal(rden[:], den[:])
    a_t = cpool.tile([P, NI], f32)
    nc.vector.tensor_mul(a_t[:], num[:], rden[:])

    # ---- out rows: out[16p + i, :] = a[p, i] * Wv -----------------------
    # Chunks of [2, 2, 4, 4, 4] blocks (smaller leading chunks would make
    # the 625 ns HWDGE descriptor generations outpace their own transfers
    # and open bubbles on the DMA engines). DVE blocks are the fastest
    # (194 ns, 2x mode); ACT/Pool take a few so chunk pacing stays ahead
    # of the transfer queue. Chunk DMAs alternate SP/ACT queues so the
    # 650 ns SEQ decodes don't pace the generations.
    out_sb = cpool.tile([P, NI, D], f32)
    out_r = out.rearrange("(p i) d -> p i d", p=P)
    chunks = [(0, 2), (2, 4), (4, 8), (8, 12), (12, 16)]
    eng_for = {2: "a", 4: "p", 6: "a", 8: "p", 10: "a", 12: "p", 14: "a"}
    for i in range(NI):
        dst = out_sb[:, i, :]
        a_col = a_t[:, i : i + 1]
        eng = eng_for.get(i, "v")
        if eng == "v":
            nc.vector.tensor_scalar_mul(dst, wv_t[:], a_col)
        elif eng == "a":
            nc.scalar.mul(dst, wv_t[:], a_col)
        else:
            nc.gpsimd.tensor_scalar_mul(dst, wv_t[:], a_col)
        for qi, (lo, hi) in enumerate(chunks):
            if i == hi - 1:
                qeng = nc.sync if qi % 2 == 0 else nc.scalar
                qeng.dma_start(out_r[:, lo:hi, :], out_sb[:, lo:hi, :])


_NC = {}


def _get_nc(repeat: int = 1):
    if repeat not in _NC:
        nc = bacc.Bacc("TRN2", target_bir_lowering=False, debug=False,
                       num_devices=N_CORES)
        build_kernel(nc, repeat)
        nc.compile()
        _NC[repeat] = nc
    return _NC[repeat]


def kernel(inputs: np.ndarray, Wq: np.ndarray, Wk: np.ndarray, Wv: np.ndarray) -> np.ndarray:
    assert inputs.shape == (N_CORES, S, 2), inputs.shape
    nc = _get_nc()
    wq = np.asarray(Wq, dtype=np.float32).reshape(P, 2)
    wk = np.asarray(Wk, dtype=np.float32).reshape(P, 2)
    wvrep = np.ascontiguousarray(
        np.broadcast_to(np.asarray(Wv, dtype=np.float32).reshape(1, D), (P, D))
    )
    in_maps = []
    perms = []
    for b in range(N_CORES):
        sig = np.asarray(inputs[b, :, 0], dtype=np.float32)
        hgt = np.asarray(inputs[b, :, 1], dtype=np.float32)
        perm = np.argsort(sig, kind="stable")
        perms.append(perm)
        xcrit = np.empty((P, 2 * NI + 4), dtype=np.float32)
        xcrit[:, 0:NI] = sig[perm].reshape(P, NI)
        xcrit[:, NI : 2 * NI] = hgt[perm].reshape(P, NI)
        xcrit[:, 2 * NI : 2 * NI + 2] = wq
        xcrit[:, 2 * NI + 2 : 2 * NI + 4] = wk
        in_maps.append({"xcrit": xcrit, "wvrep": wvrep})
    res = run_bass_kernel_spmd(nc, in_maps, core_ids=list(range(N_CORES)))
    full = np.empty((N_CORES, S, D), dtype=np.float32)
    for b in range(N_CORES):
        inv = np.empty(S, dtype=np.int64)
        inv[perms[b]] = np.arange(S)
        full[b] = res.results[b]["out"][inv]
    return full
